# revision 50
# baseline (speedup 1.0000x reference)
"""Trainium2 Bass kernel for CrossAttention (B=4, T=2048, S=4096, D=256, H=8, Dh=32).

Sharding: 8 cores = 4 batches x 2 T-halves (each core owns 1024 query rows of
one batch, all heads). No collectives: host concatenates.

Design (~1.5x faster than the v1 baseline; 466us -> ~316us traced):
- Scores are PE row-tiled: qT/kT packed per 4-head group across the 128
  partitions (head h at partitions 32h..32h+32); a chunk's K=32 score matmuls
  go to tile_position (32h, 0) and run concurrently on the PE's row-strips.
- attn@v is 4-way column-tiled (M=32 per head at tile_position (0, 32h)): all
  four heads accumulate into ONE psum bank over the 32 S-chunks; the softmax
  denominators accumulate in a second bank via M=1 ones-lhsT matmuls.
- The 33.5M softmax exps per core are split per head-pair across BOTH fixed-
  function engines: ScalarE runs the exact LUT exp on even heads while the
  Vector engine runs a Schraudolph bit-trick exp on odd heads -
  int16(x*a+b) bitcast as fp16 IS exp(x) to ~2-3% element accuracy, which
  softmax normalization averages down to ~4e-3 output error (gate 2e-2).
- PSUM budget: 3 rotating score slots [128,1024] + acc + dacc = 8 banks.
- The emission is software-pipelined by one chunk and ordered by readiness
  (scores of chunk c interleave with attn@v of c-1) so the in-order PE queue
  never head-blocks on late work.
- HAM clock-gate management: the PE clock halves (K=4/8) after any ~3.4us
  idle window and only re-warms on a ~3.4us gapless busy window. The kernel
  preloads the Exp ACT table during phase A, fires a 10-matmul "reheat burst"
  at phase/pass boundaries, and the normalization's DRAM roundtrip for
  denominator broadcast is split in two stages emitted ~10 chunks apart so no
  engine queue ever stalls long enough to re-throttle the PE.
"""

import sys

if "/opt/trn_rl_repo" not in sys.path:
    sys.path.insert(0, "/opt/trn_rl_repo")

from contextlib import ExitStack

import numpy as np

import concourse.bass as bass
import concourse.tile as tile
from concourse import bacc
from concourse import mybir
from concourse.bass_utils import run_bass_kernel_spmd

B, T, S, D, H, Dh = 4, 2048, 4096, 256, 8, 32
TL = T // 2          # 1024 query rows per core
NST = S // 128       # 32 S-chunks
SCALE = Dh ** -0.5
FP = mybir.dt.float32
F16 = mybir.dt.float16
I16 = mybir.dt.int16

LOG2E = 1.4426950408889634
SCHR_A = SCALE * 1024.0 * LOG2E     # exp(SCALE*x) ~ fp16bits(int16(x*A + B))
SCHR_B = 15360.0 - 14.8


def build_bass():
    nc = bacc.Bacc()
    ident_d = nc.declare_dram_parameter("ident", [128, 128], FP, isOutput=False)
    x_d = nc.declare_dram_parameter("x", [TL, D], FP, isOutput=False)
    ctx_d = nc.declare_dram_parameter("context", [S, D], FP, isOutput=False)
    wq_d = nc.declare_dram_parameter("w_q", [D, D], FP, isOutput=False)
    wkv_d = nc.declare_dram_parameter("w_kv", [D, 2 * D], FP, isOutput=False)
    wout_d = nc.declare_dram_parameter("w_out", [D, D], FP, isOutput=False)
    bout_d = nc.declare_dram_parameter("b_out", [1, D], FP, isOutput=False)
    out_d = nc.declare_dram_parameter("out", [TL, D], FP, isOutput=True)
    dnscr = nc.dram_tensor("dnscratch", [H, TL], FP)

    with tile.TileContext(nc) as tc, ExitStack() as ctx:
        consts = ctx.enter_context(tc.tile_pool(name="consts", bufs=1))
        persist = ctx.enter_context(tc.tile_pool(name="persist", bufs=1))

        idh = consts.tile([128, 128], F16, tag="idh", name="idh")
        bias_c = persist.tile([128, D], FP, tag="bias_c", name="bias_c")

        # fp16 weights
        wqh = [persist.tile([128, D], F16, tag=f"wqh{j}", name=f"wqh{j}") for j in range(2)]
        wkh = [persist.tile([128, D], F16, tag=f"wkh{j}", name=f"wkh{j}") for j in range(2)]
        wvh = [persist.tile([128, D], F16, tag=f"wvh{j}", name=f"wvh{j}") for j in range(2)]
        woh = [persist.tile([128, D], F16, tag=f"woh{j}", name=f"woh{j}") for j in range(2)]

        # transposed activations (d on partitions)
        xT = [persist.tile([128, TL], F16, tag=f"xT{j}", name=f"xT{j}") for j in range(2)]
        cT = [persist.tile([128, S], F16, tag=f"cT{j}", name=f"cT{j}") for j in range(2)]
        # packed projections: group g holds heads 4g..4g+3, head j at partitions 32j..
        qT = [persist.tile([128, TL], F16, tag=f"qT{g}", name=f"qT{g}") for g in range(2)]
        kT = [persist.tile([128, S], F16, tag=f"kT{g}", name=f"kT{g}") for g in range(2)]
        # v packed per chunk/head: [s=128, chunk, head, 32]
        vP = persist.tile([128, NST, H, 32], F16, tag="vP", name="vP")
        ones1 = consts.tile([128, 1], F16, tag="ones1", name="ones1")
        # normalized attention output, lhsT layout for the out projection
        outN = [persist.tile([128, TL], F16, tag=f"outN{g}", name=f"outN{g}") for g in range(2)]

        # ---------------- Phase A: load + convert + transpose + project ----
        ea = tc.tile_pool(name="early", bufs=1)
        ep = ea.__enter__()
        eps = tc.tile_pool(name="early_ps", bufs=2, space="PSUM")
        epp = eps.__enter__()

        ident_s = ep.tile([128, 128], FP, tag="ident_s", name="ident_s")
        x_all = ep.tile([128, TL // 128, D], FP, tag="x_all", name="x_all")
        c_all = ep.tile([128, NST, D], FP, tag="c_all", name="c_all")
        wstage = ep.tile([128, 6 * D], FP, tag="wstage", name="wstage")
        xh = ep.tile([128, TL // 128, D], F16, tag="xh", name="xh")
        ch = ep.tile([128, NST, D], F16, tag="ch", name="ch")

        nc.sync.dma_start(out=ident_s, in_=ident_d[:, :])
        for j in range(2):
            nc.sync.dma_start(
                out=wstage[:, j * D : j * D + D], in_=wq_d[128 * j : 128 * j + 128, :]
            )
            nc.sync.dma_start(
                out=wstage[:, (2 + 2 * j) * D : (4 + 2 * j) * D],
                in_=wkv_d[128 * j : 128 * j + 128, :],
            )
        nc.sync.dma_start(out=x_all, in_=x_d.rearrange("(t p) d -> p t d", p=128))
        ctx_r = ctx_d.rearrange("(t p) d -> p t d", p=128)
        for cc in range(4):
            nc.sync.dma_start(
                out=c_all[:, 8 * cc : 8 * cc + 8, :], in_=ctx_r[:, 8 * cc : 8 * cc + 8, :]
            )
        nc.sync.dma_start(out=bias_c, in_=bout_d[0:1, :].partition_broadcast(128))

        nc.vector.tensor_copy(idh, ident_s)
        # preload the Exp ACT table set early so the first real exp in the
        # attention loop doesn't stall the pipeline ~2.7us (which would idle
        # the PE past the HAM clock-gate's MID window and halve its clock)
        actwarm = ep.tile([128, 1], F16, tag="actwarm", name="actwarm")
        nc.scalar.activation(
            actwarm, ident_s[:, 0:1], mybir.ActivationFunctionType.Exp, scale=1.0
        )
        for j in range(2):
            nc.scalar.copy(wqh[j], wstage[:, j * D : j * D + D])
            nc.scalar.copy(wkh[j], wstage[:, (2 + 2 * j) * D : (3 + 2 * j) * D])
            nc.scalar.copy(wvh[j], wstage[:, (3 + 2 * j) * D : (4 + 2 * j) * D])
        # w_out rows 128g.. as rhs tiles [hid-part, dout]
        wos = ep.tile([128, 2, D], FP, tag="wos", name="wos")
        for g in range(2):
            nc.sync.dma_start(out=wos[:, g, :], in_=wout_d[128 * g : 128 * g + 128, :])
            nc.vector.tensor_copy(woh[g], wos[:, g, :])

        nc.vector.tensor_copy(xh, x_all)
        for cc in range(4):
            blk = ch[:, 8 * cc : 8 * cc + 8, :]
            if cc % 2 == 0:
                nc.vector.tensor_copy(blk, c_all[:, 8 * cc : 8 * cc + 8, :])
            else:
                nc.scalar.copy(blk, c_all[:, 8 * cc : 8 * cc + 8, :])

        # transposes: 4 per [128,512] psum tile, then one copy out
        tp_count = [0]

        # xT: x_all[:, t, 128j:128j+128] -> xT[j][:, 128t..]
        for j in range(2):
            for tq in range(2):  # 4 tiles per copy
                pt = epp.tile([128, 512], F16, tag="pt", name="pt", bufs=2)
                for i in range(4):
                    t = 4 * tq + i
                    nc.tensor.transpose(
                        pt[:, 128 * i : 128 * i + 128],
                        xh[:, t, 128 * j : 128 * j + 128],
                        idh,
                    )
                dst = xT[j][:, 512 * tq : 512 * tq + 512]
                if tp_count[0] % 2 == 0:
                    nc.vector.tensor_copy(dst, pt)
                else:
                    nc.scalar.copy(dst, pt)
                tp_count[0] += 1
        for j in range(2):
            for tq in range(8):
                pt = epp.tile([128, 512], F16, tag="pt", name="pt", bufs=2)
                for i in range(4):
                    t = 4 * tq + i
                    nc.tensor.transpose(
                        pt[:, 128 * i : 128 * i + 128],
                        ch[:, t, 128 * j : 128 * j + 128],
                        idh,
                    )
                dst = cT[j][:, 512 * tq : 512 * tq + 512]
                if tp_count[0] % 2 == 0:
                    nc.vector.tensor_copy(dst, pt)
                else:
                    nc.scalar.copy(dst, pt)
                tp_count[0] += 1

        # ---- projections ----
        # qT[g] = (w_q[:, 128g:128g+128])^T @ xT ; kT[g] likewise from w_kv k-part
        for g in range(2):
            for nt in range(TL // 512):
                pq = epp.tile([128, 512], FP, tag="pj", name="pq", bufs=4)
                for kj in range(2):
                    nc.tensor.matmul(
                        pq,
                        lhsT=wqh[kj][:, 128 * g : 128 * g + 128],
                        rhs=xT[kj][:, 512 * nt : 512 * nt + 512],
                        start=(kj == 0),
                        stop=(kj == 1),
                    )
                dst = qT[g][:, 512 * nt : 512 * nt + 512]
                if nt % 2 == 0:
                    nc.vector.tensor_copy(dst, pq)
                else:
                    nc.scalar.copy(dst, pq)
            for nt in range(S // 512):
                pk = epp.tile([128, 512], FP, tag="pj", name="pk", bufs=4)
                for kj in range(2):
                    nc.tensor.matmul(
                        pk,
                        lhsT=wkh[kj][:, 128 * g : 128 * g + 128],
                        rhs=cT[kj][:, 512 * nt : 512 * nt + 512],
                        start=(kj == 0),
                        stop=(kj == 1),
                    )
                dst = kT[g][:, 512 * nt : 512 * nt + 512]
                if nt % 2 == 0:
                    nc.vector.tensor_copy(dst, pk)
                else:
                    nc.scalar.copy(dst, pk)

        # v: per chunk [128s, 256(h,dv)] -> vP
        nc.vector.memset(ones1, 1.0)
        for c in range(NST):
            pv = epp.tile([128, D], FP, tag="pv", name="pv", bufs=2)
            for kj in range(2):
                nc.tensor.matmul(
                    pv,
                    lhsT=cT[kj][:, 128 * c : 128 * c + 128],
                    rhs=wvh[kj],
                    start=(kj == 0),
                    stop=(kj == 1),
                )
            dst = vP[:, c, :, :]
            src = pv.rearrange("p (h w) -> p h w", h=H)
            if c % 2 == 0:
                nc.vector.tensor_copy(dst, src)
            else:
                nc.scalar.copy(dst, src)

        eps.__exit__(None, None, None)
        ea.__exit__(None, None, None)

        # ---------------- Phase B: attention ----------------
        phb = tc.tile_pool(name="slots", bufs=3, space="PSUM")
        psl = phb.__enter__()
        phb2 = tc.tile_pool(name="accs", bufs=2, space="PSUM")
        psa = phb2.__enter__()
        atp = ctx.enter_context(tc.tile_pool(name="atp", bufs=14))
        npool = ctx.enter_context(tc.tile_pool(name="npool", bufs=2))

        # reheat burst: ~4us of gapless dense matmuls flips the HAM clock gate
        # back to K=8/8 (2.4 GHz) and bridges PE-idle windows (pool switches,
        # the normalize DMA roundtrip at pass boundaries) that would otherwise
        # re-throttle the PE for the rest of the kernel.
        def reheat(n=10):
            for _ in range(n):
                trash = psl.tile([128, 1024], FP, tag="ssc", name="trash", bufs=3)
                nc.tensor.matmul(
                    trash[:, 0:512], lhsT=cT[0][:, 0:128], rhs=cT[1][:, 0:512],
                    start=True, stop=True, skip_group_check=True,
                )

        reheat(10)

        # deferred stage-2 of the previous pass's normalization: by emitting
        # the DMA-readback-dependent ops ~10 chunks into the NEXT pass, the
        # engine queues never head-of-line block on the DRAM roundtrip.
        pending_norm = [None]

        def emit_norm_stage2():
            if pending_norm[0] is None:
                return
            g0, tp0, outU = pending_norm[0]
            pending_norm[0] = None
            rden = npool.tile([128, 512], FP, tag="rden", name="rden")
            rcp = npool.tile([128, 512], FP, tag="rcp", name="rcp")
            for h in range(4):
                nc.sync.dma_start(
                    out=rden[32 * h : 32 * h + 32, :],
                    in_=dnscr[
                        4 * g0 + h : 4 * g0 + h + 1, 512 * tp0 : 512 * tp0 + 512
                    ].partition_broadcast(32),
                )
            nc.vector.reciprocal_approx_fast(rcp, rden)
            nc.vector.tensor_mul(
                outN[g0][:, 512 * tp0 : 512 * tp0 + 512], outU, rcp
            )

        for g in range(2):
            for tp in range(2):
                # one acc bank holds all 4 heads (M=32 col tiles at 0/32/64/96);
                # a second bank accumulates the 4 softmax denominator rows.
                acc = psa.tile([128, 512], FP, tag="acc", name=f"acc{g}{tp}", bufs=1)
                dacc = psa.tile([128, 512], FP, tag="dacc", name=f"dacc{g}{tp}", bufs=1)
                # software-pipelined by one chunk: chunk c's scores/exps are
                # emitted around chunk c-1's attn@v, ordered by readiness so
                # the in-order PE queue never head-blocks on late work.
                prev_at = [None]

                def score_mm(slot_ap, c, h, pos):
                    nc.tensor.matmul(
                        slot_ap,
                        lhsT=kT[g][32 * h : 32 * h + 32, 128 * c : 128 * c + 128],
                        rhs=qT[g][32 * h : 32 * h + 32, 512 * tp : 512 * tp + 512],
                        start=True,
                        stop=True,
                        tile_position=(32 * h, 0),
                        skip_group_check=True,
                    )

                def emit_scores(c, j):
                    # head-pair 2j,2j+1 -> one 2-bank slot; exp split across
                    # BOTH engines (scalar: head 2j exact, DVE: head 2j+1
                    # schraudolph) so the slot frees in ~0.72us not ~1.2us.
                    slot = psl.tile([128, 1024], FP, tag="ssc", name="ssc", bufs=3)
                    for hh in range(2):
                        h = 2 * j + hh
                        score_mm(slot[:, 512 * hh : 512 * hh + 512], c, h, h)
                    nc.scalar.activation(
                        at_c[:, 2 * j, :],
                        slot[:, 0:512],
                        mybir.ActivationFunctionType.Exp,
                        scale=SCALE,
                    )
                    nc.vector.tensor_scalar(
                        out=at_c[:, 2 * j + 1, :].bitcast(I16),
                        in0=slot[:, 512:1024],
                        scalar1=SCHR_A,
                        scalar2=SCHR_B,
                        op0=mybir.AluOpType.mult,
                        op1=mybir.AluOpType.add,
                    )

                def emit_attnv1(c, at_p, h):
                    nc.tensor.matmul(
                        acc[32 * h : 32 * h + 32, :],
                        lhsT=vP[:, c, 4 * g + h, :],
                        rhs=at_p[:, h, :],
                        start=(c == 0),
                        stop=(c == NST - 1),
                        tile_position=(0, 32 * h),
                        skip_group_check=True,
                    )
                    nc.tensor.matmul(
                        dacc[32 * h : 32 * h + 1, :],
                        lhsT=ones1,
                        rhs=at_p[:, h, :],
                        start=(c == 0),
                        stop=(c == NST - 1),
                        tile_position=(0, 32 * h),
                        skip_group_check=True,
                    )

                for c in range(NST):
                    if c == 10:
                        emit_norm_stage2()
                    at_c = atp.tile([128, 4, 512], F16, tag="at", name="at")
                    emit_scores(c, 0)
                    if prev_at[0] is not None:
                        emit_attnv1(c - 1, prev_at[0], 0)
                        emit_attnv1(c - 1, prev_at[0], 1)
                    emit_scores(c, 1)
                    if prev_at[0] is not None:
                        emit_attnv1(c - 1, prev_at[0], 2)
                        emit_attnv1(c - 1, prev_at[0], 3)
                    prev_at[0] = at_c
                for h in range(4):
                    emit_attnv1(NST - 1, prev_at[0], h)


                # bridge the acc-drain + denominator DMA roundtrip with PE work
                reheat(10)

                # normalize stage 1: drain acc + ship denominators to DRAM
                outU = npool.tile([128, 512], F16, tag="outU", name="outU", bufs=2)
                nc.vector.tensor_copy(outU, acc)
                for h in range(4):
                    dnt = npool.tile([1, 512], FP, tag=f"dnt{h}", name=f"dnt{h}")
                    nc.vector.tensor_copy(dnt, dacc[32 * h : 32 * h + 1, :])
                    nc.sync.dma_start(
                        out=dnscr[4 * g + h : 4 * g + h + 1, 512 * tp : 512 * tp + 512],
                        in_=dnt,
                    )
                pending_norm[0] = (g, tp, outU)

        emit_norm_stage2()

        phb2.__exit__(None, None, None)
        phb.__exit__(None, None, None)

        # ---------------- Phase C: output projection ----------------
        fps = ctx.enter_context(tc.tile_pool(name="fin_ps", bufs=3, space="PSUM"))
        fsb = ctx.enter_context(tc.tile_pool(name="fin_sb", bufs=3))
        for tt in range(TL // 128):
            fin = fps.tile([128, D], FP, tag="fin", name="fin")
            for g in range(2):
                nc.tensor.matmul(
                    fin,
                    lhsT=outN[g][:, 128 * tt : 128 * tt + 128],
                    rhs=woh[g],
                    start=(g == 0),
                    stop=(g == 1),
                )
            outs = fsb.tile([128, D], FP, tag="outs", name="outs")
            nc.vector.tensor_add(outs, fin, bias_c)
            nc.sync.dma_start(out=out_d[128 * tt : 128 * tt + 128, :], in_=outs)

    nc.compile()
    return nc


_NC = None


def kernel(**inputs):
    global _NC
    x = np.ascontiguousarray(inputs["x"], dtype=np.float32)
    context = np.ascontiguousarray(inputs["context"], dtype=np.float32)
    w_q = np.ascontiguousarray(inputs["w_q"], dtype=np.float32)
    w_kv = np.ascontiguousarray(inputs["w_kv"], dtype=np.float32)
    w_out = np.ascontiguousarray(inputs["w_out"], dtype=np.float32)
    b_out = np.ascontiguousarray(inputs["b_out"], dtype=np.float32).reshape(1, D)

    if _NC is None:
        _NC = build_bass()
    nc = _NC

    in_maps = []
    for c in range(8):
        b, half = c // 2, c % 2
        in_maps.append(
            {
                "ident": np.eye(128, dtype=np.float32),
                "x": np.ascontiguousarray(x[b, TL * half : TL * half + TL, :]),
                "context": np.ascontiguousarray(context[b]),
                "w_q": w_q,
                "w_kv": w_kv,
                "w_out": w_out,
                "b_out": b_out,
            }
        )
    res = run_bass_kernel_spmd(nc, in_maps, core_ids=list(range(8)))
    out = np.empty((B, T, D), dtype=np.float32)
    for c in range(8):
        b, half = c // 2, c % 2
        out[b, TL * half : TL * half + TL, :] = res.results[c]["out"]
    return out


if __name__ == "__main__":
    rng = np.random.default_rng(0)
    ins = {
        "x": rng.standard_normal((B, T, D), dtype=np.float32),
        "context": rng.standard_normal((B, S, D), dtype=np.float32),
        "w_q": rng.standard_normal((D, D), dtype=np.float32) * D**-0.5,
        "w_kv": rng.standard_normal((D, 2 * D), dtype=np.float32) * D**-0.5,
        "w_out": rng.standard_normal((D, D), dtype=np.float32) * D**-0.5,
        "b_out": rng.standard_normal((D,), dtype=np.float32) * 0.01,
    }
    out = kernel(**ins)
    print(out.shape, out.dtype, np.abs(out).mean())


# revision 51
# speedup vs baseline: 1.1131x; 1.1131x over previous
"""Trainium2 Bass kernel for CrossAttention (B=4, T=2048, S=4096, D=256, H=8, Dh=32).

Sharding: 8 cores = 4 batches x 2 T-halves (each core owns 1024 query rows of
one batch, all heads). No collectives: host concatenates.

Design (~1.5x faster than the v1 baseline; 466us -> ~316us traced):
- Scores are PE row-tiled: qT/kT packed per 4-head group across the 128
  partitions (head h at partitions 32h..32h+32); a chunk's K=32 score matmuls
  go to tile_position (32h, 0) and run concurrently on the PE's row-strips.
- attn@v is 4-way column-tiled (M=32 per head at tile_position (0, 32h)): all
  four heads accumulate into ONE psum bank over the 32 S-chunks; the softmax
  denominators accumulate in a second bank via M=1 ones-lhsT matmuls.
- The 33.5M softmax exps per core are split per head-pair across BOTH fixed-
  function engines: ScalarE runs the exact LUT exp on even heads while the
  Vector engine runs a Schraudolph bit-trick exp on odd heads -
  int16(x*a+b) bitcast as fp16 IS exp(x) to ~2-3% element accuracy, which
  softmax normalization averages down to ~4e-3 output error (gate 2e-2).
- PSUM budget: 3 rotating score slots [128,1024] + acc + dacc = 8 banks.
- The emission is software-pipelined by one chunk and ordered by readiness
  (scores of chunk c interleave with attn@v of c-1) so the in-order PE queue
  never head-blocks on late work.
- HAM clock-gate management: the PE clock halves (K=4/8) after any ~3.4us
  idle window and only re-warms on a ~3.4us gapless busy window. The kernel
  preloads the Exp ACT table during phase A, fires a 10-matmul "reheat burst"
  at phase/pass boundaries, and the normalization's DRAM roundtrip for
  denominator broadcast is split in two stages emitted ~10 chunks apart so no
  engine queue ever stalls long enough to re-throttle the PE.
"""

import sys

if "/opt/trn_rl_repo" not in sys.path:
    sys.path.insert(0, "/opt/trn_rl_repo")

from contextlib import ExitStack

import numpy as np

import concourse.bass as bass
import concourse.tile as tile
from concourse import bacc
from concourse import mybir
from concourse.bass_utils import run_bass_kernel_spmd

B, T, S, D, H, Dh = 4, 2048, 4096, 256, 8, 32
TL = T // 2          # 1024 query rows per core
NST = S // 128       # 32 S-chunks
SCALE = Dh ** -0.5
FP = mybir.dt.float32
F16 = mybir.dt.float16
I16 = mybir.dt.int16

LOG2E = 1.4426950408889634
SCHR_A = SCALE * 1024.0 * LOG2E     # exp(SCALE*x) ~ fp16bits(int16(x*A + B))
SCHR_B = 15360.0 - 14.8


def build_bass():
    nc = bacc.Bacc()
    ident_d = nc.declare_dram_parameter("ident", [128, 128], FP, isOutput=False)
    x_d = nc.declare_dram_parameter("x", [TL, D], FP, isOutput=False)
    ctx_d = nc.declare_dram_parameter("context", [S, D], FP, isOutput=False)
    wq_d = nc.declare_dram_parameter("w_q", [D, D], FP, isOutput=False)
    wkv_d = nc.declare_dram_parameter("w_kv", [D, 2 * D], FP, isOutput=False)
    wout_d = nc.declare_dram_parameter("w_out", [D, D], FP, isOutput=False)
    bout_d = nc.declare_dram_parameter("b_out", [1, D], FP, isOutput=False)
    out_d = nc.declare_dram_parameter("out", [TL, D], FP, isOutput=True)
    dnscr = nc.dram_tensor("dnscratch", [H, TL], FP)

    with tile.TileContext(nc) as tc, ExitStack() as ctx:
        consts = ctx.enter_context(tc.tile_pool(name="consts", bufs=1))
        persist = ctx.enter_context(tc.tile_pool(name="persist", bufs=1))

        idh = consts.tile([128, 128], F16, tag="idh", name="idh")
        bias_c = persist.tile([128, D], FP, tag="bias_c", name="bias_c")

        # fp16 weights
        wqh = [persist.tile([128, D], F16, tag=f"wqh{j}", name=f"wqh{j}") for j in range(2)]
        wkh = [persist.tile([128, D], F16, tag=f"wkh{j}", name=f"wkh{j}") for j in range(2)]
        wvh = [persist.tile([128, D], F16, tag=f"wvh{j}", name=f"wvh{j}") for j in range(2)]
        woh = [persist.tile([128, D], F16, tag=f"woh{j}", name=f"woh{j}") for j in range(2)]

        # transposed activations (d on partitions)
        xT = [persist.tile([128, TL], F16, tag=f"xT{j}", name=f"xT{j}") for j in range(2)]
        cT = [persist.tile([128, S], F16, tag=f"cT{j}", name=f"cT{j}") for j in range(2)]
        # packed projections: group g holds heads 4g..4g+3, head j at partitions 32j..
        qT = [persist.tile([128, TL], F16, tag=f"qT{g}", name=f"qT{g}") for g in range(2)]
        kT = [persist.tile([128, S], F16, tag=f"kT{g}", name=f"kT{g}") for g in range(2)]
        # v packed per chunk/head: [s=128, chunk, head, 32]
        vP = persist.tile([128, NST, H, 32], F16, tag="vP", name="vP")
        ones1 = consts.tile([128, 1], F16, tag="ones1", name="ones1")
        # normalized attention output, lhsT layout for the out projection
        outN = [persist.tile([128, TL], F16, tag=f"outN{g}", name=f"outN{g}") for g in range(2)]

        # ---------------- Phase A: load + convert + transpose + project ----
        ea = tc.tile_pool(name="early", bufs=1)
        ep = ea.__enter__()
        eps = tc.tile_pool(name="early_ps", bufs=2, space="PSUM")
        epp = eps.__enter__()

        ident_s = ep.tile([128, 128], FP, tag="ident_s", name="ident_s")
        x_all = ep.tile([128, TL // 128, D], FP, tag="x_all", name="x_all")
        c_all = ep.tile([128, NST, D], FP, tag="c_all", name="c_all")
        wstage = ep.tile([128, 6 * D], FP, tag="wstage", name="wstage")
        xh = ep.tile([128, TL // 128, D], F16, tag="xh", name="xh")
        ch = ep.tile([128, NST, D], F16, tag="ch", name="ch")

        nc.sync.dma_start(out=ident_s, in_=ident_d[:, :])
        x_r = x_d.rearrange("(t p) d -> p t d", p=128)
        for xb in range(2):
            nc.sync.dma_start(
                out=x_all[:, 4 * xb : 4 * xb + 4, :], in_=x_r[:, 4 * xb : 4 * xb + 4, :]
            )
        for j in range(2):
            nc.sync.dma_start(
                out=wstage[:, j * D : j * D + D], in_=wq_d[128 * j : 128 * j + 128, :]
            )
            nc.sync.dma_start(
                out=wstage[:, (2 + 2 * j) * D : (4 + 2 * j) * D],
                in_=wkv_d[128 * j : 128 * j + 128, :],
            )
        ctx_r = ctx_d.rearrange("(t p) d -> p t d", p=128)
        for cc in range(4):
            nc.sync.dma_start(
                out=c_all[:, 8 * cc : 8 * cc + 8, :], in_=ctx_r[:, 8 * cc : 8 * cc + 8, :]
            )
        nc.sync.dma_start(out=bias_c, in_=bout_d[0:1, :].partition_broadcast(128))

        nc.vector.tensor_copy(idh, ident_s)
        # preload the Exp ACT table set early so the first real exp in the
        # attention loop doesn't stall the pipeline ~2.7us (which would idle
        # the PE past the HAM clock-gate's MID window and halve its clock)
        actwarm = ep.tile([128, 1], F16, tag="actwarm", name="actwarm")
        nc.scalar.activation(
            actwarm, ident_s[:, 0:1], mybir.ActivationFunctionType.Exp, scale=1.0
        )
        for j in range(2):
            nc.scalar.copy(wqh[j], wstage[:, j * D : j * D + D])
            nc.scalar.copy(wkh[j], wstage[:, (2 + 2 * j) * D : (3 + 2 * j) * D])
            nc.scalar.copy(wvh[j], wstage[:, (3 + 2 * j) * D : (4 + 2 * j) * D])
        # w_out rows 128g.. as rhs tiles [hid-part, dout]
        wos = ep.tile([128, 2, D], FP, tag="wos", name="wos")
        for g in range(2):
            nc.sync.dma_start(out=wos[:, g, :], in_=wout_d[128 * g : 128 * g + 128, :])
            nc.vector.tensor_copy(woh[g], wos[:, g, :])

        for xb in range(2):
            nc.vector.tensor_copy(
                xh[:, 4 * xb : 4 * xb + 4, :], x_all[:, 4 * xb : 4 * xb + 4, :]
            )
        for cc in range(4):
            blk = ch[:, 8 * cc : 8 * cc + 8, :]
            if cc % 2 == 0:
                nc.vector.tensor_copy(blk, c_all[:, 8 * cc : 8 * cc + 8, :])
            else:
                nc.scalar.copy(blk, c_all[:, 8 * cc : 8 * cc + 8, :])

        # transposes: 4 per [128,512] psum tile, then one copy out
        tp_count = [0]

        # xT: x_all[:, t, 128j:128j+128] -> xT[j][:, 128t..]
        for tq in range(2):  # 4 tiles per copy
            for j in range(2):
                pt = epp.tile([128, 512], F16, tag="pt", name="pt", bufs=2)
                for i in range(4):
                    t = 4 * tq + i
                    nc.tensor.transpose(
                        pt[:, 128 * i : 128 * i + 128],
                        xh[:, t, 128 * j : 128 * j + 128],
                        idh,
                    )
                dst = xT[j][:, 512 * tq : 512 * tq + 512]
                if tp_count[0] % 2 == 0:
                    nc.vector.tensor_copy(dst, pt)
                else:
                    nc.scalar.copy(dst, pt)
                tp_count[0] += 1
        for j in range(2):
            for tq in range(8):
                pt = epp.tile([128, 512], F16, tag="pt", name="pt", bufs=2)
                for i in range(4):
                    t = 4 * tq + i
                    nc.tensor.transpose(
                        pt[:, 128 * i : 128 * i + 128],
                        ch[:, t, 128 * j : 128 * j + 128],
                        idh,
                    )
                dst = cT[j][:, 512 * tq : 512 * tq + 512]
                if tp_count[0] % 2 == 0:
                    nc.vector.tensor_copy(dst, pt)
                else:
                    nc.scalar.copy(dst, pt)
                tp_count[0] += 1

        # ---- projections ----
        # qT[g] = (w_q[:, 128g:128g+128])^T @ xT ; kT[g] likewise from w_kv k-part
        for g in range(2):
            for nt in range(TL // 512):
                pq = epp.tile([128, 512], FP, tag="pj", name="pq", bufs=4)
                for kj in range(2):
                    nc.tensor.matmul(
                        pq,
                        lhsT=wqh[kj][:, 128 * g : 128 * g + 128],
                        rhs=xT[kj][:, 512 * nt : 512 * nt + 512],
                        start=(kj == 0),
                        stop=(kj == 1),
                    )
                dst = qT[g][:, 512 * nt : 512 * nt + 512]
                if nt % 2 == 0:
                    nc.vector.tensor_copy(dst, pq)
                else:
                    nc.scalar.copy(dst, pq)
            for nt in range(S // 512):
                pk = epp.tile([128, 512], FP, tag="pj", name="pk", bufs=4)
                for kj in range(2):
                    nc.tensor.matmul(
                        pk,
                        lhsT=wkh[kj][:, 128 * g : 128 * g + 128],
                        rhs=cT[kj][:, 512 * nt : 512 * nt + 512],
                        start=(kj == 0),
                        stop=(kj == 1),
                    )
                dst = kT[g][:, 512 * nt : 512 * nt + 512]
                if nt % 2 == 0:
                    nc.vector.tensor_copy(dst, pk)
                else:
                    nc.scalar.copy(dst, pk)

        # v: per chunk [128s, 256(h,dv)] -> vP
        nc.vector.memset(ones1, 1.0)
        for c in range(NST):
            pv = epp.tile([128, D], FP, tag="pv", name="pv", bufs=2)
            for kj in range(2):
                nc.tensor.matmul(
                    pv,
                    lhsT=cT[kj][:, 128 * c : 128 * c + 128],
                    rhs=wvh[kj],
                    start=(kj == 0),
                    stop=(kj == 1),
                )
            dst = vP[:, c, :, :]
            src = pv.rearrange("p (h w) -> p h w", h=H)
            if c % 2 == 0:
                nc.vector.tensor_copy(dst, src)
            else:
                nc.scalar.copy(dst, src)

        eps.__exit__(None, None, None)
        ea.__exit__(None, None, None)

        # ---------------- Phase B: attention ----------------
        phb = tc.tile_pool(name="slots", bufs=3, space="PSUM")
        psl = phb.__enter__()
        phb2 = tc.tile_pool(name="accs", bufs=2, space="PSUM")
        psa = phb2.__enter__()
        atp = ctx.enter_context(tc.tile_pool(name="atp", bufs=14))
        npool = ctx.enter_context(tc.tile_pool(name="npool", bufs=2))

        # reheat burst: ~4us of gapless dense matmuls flips the HAM clock gate
        # back to K=8/8 (2.4 GHz) and bridges PE-idle windows (pool switches,
        # the normalize DMA roundtrip at pass boundaries) that would otherwise
        # re-throttle the PE for the rest of the kernel.
        def reheat(n=10):
            for _ in range(n):
                trash = psl.tile([128, 1024], FP, tag="ssc", name="trash", bufs=3)
                nc.tensor.matmul(
                    trash[:, 0:512], lhsT=cT[0][:, 0:128], rhs=cT[1][:, 0:512],
                    start=True, stop=True, skip_group_check=True,
                )

        reheat(10)

        # deferred stage-2 of the previous pass's normalization: by emitting
        # the DMA-readback-dependent ops ~10 chunks into the NEXT pass, the
        # engine queues never head-of-line block on the DRAM roundtrip.
        pending_norm = [None]

        def emit_norm_stage2():
            if pending_norm[0] is None:
                return
            g0, tp0, outU = pending_norm[0]
            pending_norm[0] = None
            rden = npool.tile([128, 512], FP, tag="rden", name="rden")
            rcp = npool.tile([128, 512], FP, tag="rcp", name="rcp")
            for h in range(4):
                nc.sync.dma_start(
                    out=rden[32 * h : 32 * h + 32, :],
                    in_=dnscr[
                        4 * g0 + h : 4 * g0 + h + 1, 512 * tp0 : 512 * tp0 + 512
                    ].partition_broadcast(32),
                )
            nc.vector.reciprocal_approx_fast(rcp, rden)
            nc.vector.tensor_mul(
                outN[g0][:, 512 * tp0 : 512 * tp0 + 512], outU, rcp
            )

        for g in range(2):
            for tp in range(2):
                # one acc bank holds all 4 heads (M=32 col tiles at 0/32/64/96);
                # a second bank accumulates the 4 softmax denominator rows.
                acc = psa.tile([128, 512], FP, tag="acc", name=f"acc{g}{tp}", bufs=1)
                dacc = psa.tile([128, 512], FP, tag="dacc", name=f"dacc{g}{tp}", bufs=1)
                # software-pipelined by one chunk: chunk c's scores/exps are
                # emitted around chunk c-1's attn@v, ordered by readiness so
                # the in-order PE queue never head-blocks on late work.
                prev_at = [None]

                def score_mm(slot_ap, c, h, pos):
                    nc.tensor.matmul(
                        slot_ap,
                        lhsT=kT[g][32 * h : 32 * h + 32, 128 * c : 128 * c + 128],
                        rhs=qT[g][32 * h : 32 * h + 32, 512 * tp : 512 * tp + 512],
                        start=True,
                        stop=True,
                        tile_position=(32 * h, 0),
                        skip_group_check=True,
                    )

                def emit_scores(c, j):
                    # head-pair 2j,2j+1 -> one 2-bank slot; exp split across
                    # BOTH engines (scalar: head 2j exact, DVE: head 2j+1
                    # schraudolph) so the slot frees in ~0.72us not ~1.2us.
                    slot = psl.tile([128, 1024], FP, tag="ssc", name="ssc", bufs=3)
                    for hh in range(2):
                        h = 2 * j + hh
                        score_mm(slot[:, 512 * hh : 512 * hh + 512], c, h, h)
                    nc.scalar.activation(
                        at_c[:, 2 * j, :],
                        slot[:, 0:512],
                        mybir.ActivationFunctionType.Exp,
                        scale=SCALE,
                    )
                    nc.vector.tensor_scalar(
                        out=at_c[:, 2 * j + 1, :].bitcast(I16),
                        in0=slot[:, 512:1024],
                        scalar1=SCHR_A,
                        scalar2=SCHR_B,
                        op0=mybir.AluOpType.mult,
                        op1=mybir.AluOpType.add,
                    )

                def emit_attnv1(c, at_p, h):
                    nc.tensor.matmul(
                        acc[32 * h : 32 * h + 32, :],
                        lhsT=vP[:, c, 4 * g + h, :],
                        rhs=at_p[:, h, :],
                        start=(c == 0),
                        stop=(c == NST - 1),
                        tile_position=(0, 32 * h),
                        skip_group_check=True,
                    )
                    nc.tensor.matmul(
                        dacc[32 * h : 32 * h + 1, :],
                        lhsT=ones1,
                        rhs=at_p[:, h, :],
                        start=(c == 0),
                        stop=(c == NST - 1),
                        tile_position=(0, 32 * h),
                        skip_group_check=True,
                    )

                for c in range(NST):
                    if c == 10:
                        emit_norm_stage2()
                    at_c = atp.tile([128, 4, 512], F16, tag="at", name="at")
                    emit_scores(c, 0)
                    if prev_at[0] is not None:
                        emit_attnv1(c - 1, prev_at[0], 0)
                        emit_attnv1(c - 1, prev_at[0], 1)
                    emit_scores(c, 1)
                    if prev_at[0] is not None:
                        emit_attnv1(c - 1, prev_at[0], 2)
                        emit_attnv1(c - 1, prev_at[0], 3)
                    prev_at[0] = at_c
                for h in range(4):
                    emit_attnv1(NST - 1, prev_at[0], h)


                # bridge the acc-drain + denominator DMA roundtrip with PE work
                reheat(6)

                # normalize stage 1: drain acc + ship denominators to DRAM
                outU = npool.tile([128, 512], F16, tag="outU", name="outU", bufs=2)
                nc.vector.tensor_copy(outU, acc)
                for h in range(4):
                    dnt = npool.tile([1, 512], FP, tag=f"dnt{h}", name=f"dnt{h}")
                    nc.vector.tensor_copy(dnt, dacc[32 * h : 32 * h + 1, :])
                    nc.sync.dma_start(
                        out=dnscr[4 * g + h : 4 * g + h + 1, 512 * tp : 512 * tp + 512],
                        in_=dnt,
                    )
                pending_norm[0] = (g, tp, outU)

        emit_norm_stage2()

        phb2.__exit__(None, None, None)
        phb.__exit__(None, None, None)

        # ---------------- Phase C: output projection ----------------
        fps = ctx.enter_context(tc.tile_pool(name="fin_ps", bufs=3, space="PSUM"))
        fsb = ctx.enter_context(tc.tile_pool(name="fin_sb", bufs=3))
        for tt in range(TL // 128):
            fin = fps.tile([128, D], FP, tag="fin", name="fin")
            for g in range(2):
                nc.tensor.matmul(
                    fin,
                    lhsT=outN[g][:, 128 * tt : 128 * tt + 128],
                    rhs=woh[g],
                    start=(g == 0),
                    stop=(g == 1),
                )
            outs = fsb.tile([128, D], FP, tag="outs", name="outs")
            nc.vector.tensor_add(outs, fin, bias_c)
            nc.sync.dma_start(out=out_d[128 * tt : 128 * tt + 128, :], in_=outs)

    nc.compile()
    return nc


_NC = None


def kernel(**inputs):
    global _NC
    x = np.ascontiguousarray(inputs["x"], dtype=np.float32)
    context = np.ascontiguousarray(inputs["context"], dtype=np.float32)
    w_q = np.ascontiguousarray(inputs["w_q"], dtype=np.float32)
    w_kv = np.ascontiguousarray(inputs["w_kv"], dtype=np.float32)
    w_out = np.ascontiguousarray(inputs["w_out"], dtype=np.float32)
    b_out = np.ascontiguousarray(inputs["b_out"], dtype=np.float32).reshape(1, D)

    if _NC is None:
        _NC = build_bass()
    nc = _NC

    in_maps = []
    for c in range(8):
        b, half = c // 2, c % 2
        in_maps.append(
            {
                "ident": np.eye(128, dtype=np.float32),
                "x": np.ascontiguousarray(x[b, TL * half : TL * half + TL, :]),
                "context": np.ascontiguousarray(context[b]),
                "w_q": w_q,
                "w_kv": w_kv,
                "w_out": w_out,
                "b_out": b_out,
            }
        )
    res = run_bass_kernel_spmd(nc, in_maps, core_ids=list(range(8)))
    out = np.empty((B, T, D), dtype=np.float32)
    for c in range(8):
        b, half = c // 2, c % 2
        out[b, TL * half : TL * half + TL, :] = res.results[c]["out"]
    return out


if __name__ == "__main__":
    rng = np.random.default_rng(0)
    ins = {
        "x": rng.standard_normal((B, T, D), dtype=np.float32),
        "context": rng.standard_normal((B, S, D), dtype=np.float32),
        "w_q": rng.standard_normal((D, D), dtype=np.float32) * D**-0.5,
        "w_kv": rng.standard_normal((D, 2 * D), dtype=np.float32) * D**-0.5,
        "w_out": rng.standard_normal((D, D), dtype=np.float32) * D**-0.5,
        "b_out": rng.standard_normal((D,), dtype=np.float32) * 0.01,
    }
    out = kernel(**ins)
    print(out.shape, out.dtype, np.abs(out).mean())


# revision 52
# speedup vs baseline: 1.1369x; 1.0214x over previous
"""Trainium2 Bass kernel for CrossAttention (B=4, T=2048, S=4096, D=256, H=8, Dh=32).

Sharding: 8 cores = 4 batches x 2 T-halves (each core owns 1024 query rows of
one batch, all heads). No collectives: host concatenates.

Design (~1.5x faster than the v1 baseline; 466us -> ~316us traced):
- Scores are PE row-tiled: qT/kT packed per 4-head group across the 128
  partitions (head h at partitions 32h..32h+32); a chunk's K=32 score matmuls
  go to tile_position (32h, 0) and run concurrently on the PE's row-strips.
- attn@v is 4-way column-tiled (M=32 per head at tile_position (0, 32h)): all
  four heads accumulate into ONE psum bank over the 32 S-chunks; the softmax
  denominators accumulate in a second bank via M=1 ones-lhsT matmuls.
- The 33.5M softmax exps per core are split per head-pair across BOTH fixed-
  function engines: ScalarE runs the exact LUT exp on even heads while the
  Vector engine runs a Schraudolph bit-trick exp on odd heads -
  int16(x*a+b) bitcast as fp16 IS exp(x) to ~2-3% element accuracy, which
  softmax normalization averages down to ~4e-3 output error (gate 2e-2).
- PSUM budget: 3 rotating score slots [128,1024] + acc + dacc = 8 banks.
- The emission is software-pipelined by one chunk and ordered by readiness
  (scores of chunk c interleave with attn@v of c-1) so the in-order PE queue
  never head-blocks on late work.
- HAM clock-gate management: the PE clock halves (K=4/8) after any ~3.4us
  idle window and only re-warms on a ~3.4us gapless busy window. The kernel
  preloads the Exp ACT table during phase A, fires a 10-matmul "reheat burst"
  at phase/pass boundaries, and the normalization's DRAM roundtrip for
  denominator broadcast is split in two stages emitted ~10 chunks apart so no
  engine queue ever stalls long enough to re-throttle the PE.
"""

import sys

if "/opt/trn_rl_repo" not in sys.path:
    sys.path.insert(0, "/opt/trn_rl_repo")

from contextlib import ExitStack

import numpy as np

import concourse.bass as bass
import concourse.tile as tile
from concourse import bacc
from concourse import mybir
from concourse.bass_utils import run_bass_kernel_spmd

B, T, S, D, H, Dh = 4, 2048, 4096, 256, 8, 32
TL = T // 2          # 1024 query rows per core
NST = S // 128       # 32 S-chunks
SCALE = Dh ** -0.5
FP = mybir.dt.float32
F16 = mybir.dt.float16
I16 = mybir.dt.int16

LOG2E = 1.4426950408889634
SCHR_A = SCALE * 1024.0 * LOG2E     # exp(SCALE*x) ~ fp16bits(int16(x*A + B))
SCHR_B = 15360.0 - 14.8


def build_bass():
    nc = bacc.Bacc()
    ident_d = nc.declare_dram_parameter("ident", [128, 128], FP, isOutput=False)
    x_d = nc.declare_dram_parameter("x", [TL, D], FP, isOutput=False)
    ctx_d = nc.declare_dram_parameter("context", [S, D], FP, isOutput=False)
    wq_d = nc.declare_dram_parameter("w_q", [D, D], FP, isOutput=False)
    wkv_d = nc.declare_dram_parameter("w_kv", [D, 2 * D], FP, isOutput=False)
    wout_d = nc.declare_dram_parameter("w_out", [D, D], FP, isOutput=False)
    bout_d = nc.declare_dram_parameter("b_out", [1, D], FP, isOutput=False)
    out_d = nc.declare_dram_parameter("out", [TL, D], FP, isOutput=True)
    dnscr = nc.dram_tensor("dnscratch", [H, TL], FP)

    with tile.TileContext(nc) as tc, ExitStack() as ctx:
        consts = ctx.enter_context(tc.tile_pool(name="consts", bufs=1))
        persist = ctx.enter_context(tc.tile_pool(name="persist", bufs=1))

        idh = consts.tile([128, 128], F16, tag="idh", name="idh")
        bias_c = persist.tile([128, D], FP, tag="bias_c", name="bias_c")

        # fp16 weights
        wqh = [persist.tile([128, D], F16, tag=f"wqh{j}", name=f"wqh{j}") for j in range(2)]
        wkh = [persist.tile([128, D], F16, tag=f"wkh{j}", name=f"wkh{j}") for j in range(2)]
        wvh = [persist.tile([128, D], F16, tag=f"wvh{j}", name=f"wvh{j}") for j in range(2)]
        woh = [persist.tile([128, D], F16, tag=f"woh{j}", name=f"woh{j}") for j in range(2)]

        # transposed activations (d on partitions)
        xT = [persist.tile([128, TL], F16, tag=f"xT{j}", name=f"xT{j}") for j in range(2)]
        cT = [persist.tile([128, S], F16, tag=f"cT{j}", name=f"cT{j}") for j in range(2)]
        # packed projections: group g holds heads 4g..4g+3, head j at partitions 32j..
        qT = [persist.tile([128, TL], F16, tag=f"qT{g}", name=f"qT{g}") for g in range(2)]
        kT = [persist.tile([128, S], F16, tag=f"kT{g}", name=f"kT{g}") for g in range(2)]
        # v packed per chunk/head: [s=128, chunk, head, 32]
        vP = persist.tile([128, NST, H, 32], F16, tag="vP", name="vP")
        ones1 = consts.tile([128, 1], F16, tag="ones1", name="ones1")
        # normalized attention output, lhsT layout for the out projection
        outN = [persist.tile([128, TL], F16, tag=f"outN{g}", name=f"outN{g}") for g in range(2)]

        # ---------------- Phase A: load + convert + transpose + project ----
        ea = tc.tile_pool(name="early", bufs=1)
        ep = ea.__enter__()
        eps = tc.tile_pool(name="early_ps", bufs=2, space="PSUM")
        epp = eps.__enter__()

        ident_s = ep.tile([128, 128], FP, tag="ident_s", name="ident_s")
        x_all = ep.tile([128, TL // 128, D], FP, tag="x_all", name="x_all")
        c_all = ep.tile([128, NST, D], FP, tag="c_all", name="c_all")
        wstage = ep.tile([128, 6 * D], FP, tag="wstage", name="wstage")
        xh = ep.tile([128, TL // 128, D], F16, tag="xh", name="xh")
        ch = ep.tile([128, NST, D], F16, tag="ch", name="ch")

        nc.sync.dma_start(out=ident_s, in_=ident_d[:, :])
        x_r = x_d.rearrange("(t p) d -> p t d", p=128)
        for xb in range(2):
            nc.sync.dma_start(
                out=x_all[:, 4 * xb : 4 * xb + 4, :], in_=x_r[:, 4 * xb : 4 * xb + 4, :]
            )
        for j in range(2):
            nc.sync.dma_start(
                out=wstage[:, j * D : j * D + D], in_=wq_d[128 * j : 128 * j + 128, :]
            )
            nc.sync.dma_start(
                out=wstage[:, (2 + 2 * j) * D : (4 + 2 * j) * D],
                in_=wkv_d[128 * j : 128 * j + 128, :],
            )
        ctx_r = ctx_d.rearrange("(t p) d -> p t d", p=128)
        for cc in range(4):
            nc.sync.dma_start(
                out=c_all[:, 8 * cc : 8 * cc + 8, :], in_=ctx_r[:, 8 * cc : 8 * cc + 8, :]
            )
        nc.sync.dma_start(out=bias_c, in_=bout_d[0:1, :].partition_broadcast(128))

        nc.vector.tensor_copy(idh, ident_s)
        # preload the Exp ACT table set early so the first real exp in the
        # attention loop doesn't stall the pipeline ~2.7us (which would idle
        # the PE past the HAM clock-gate's MID window and halve its clock)
        actwarm = ep.tile([128, 1], F16, tag="actwarm", name="actwarm")
        nc.scalar.activation(
            actwarm, ident_s[:, 0:1], mybir.ActivationFunctionType.Exp, scale=1.0
        )
        for j in range(2):
            nc.scalar.copy(wqh[j], wstage[:, j * D : j * D + D])
            nc.scalar.copy(wkh[j], wstage[:, (2 + 2 * j) * D : (3 + 2 * j) * D])
            nc.scalar.copy(wvh[j], wstage[:, (3 + 2 * j) * D : (4 + 2 * j) * D])
        # w_out rows 128g.. as rhs tiles [hid-part, dout]
        wos = ep.tile([128, 2, D], FP, tag="wos", name="wos")
        for g in range(2):
            nc.sync.dma_start(out=wos[:, g, :], in_=wout_d[128 * g : 128 * g + 128, :])
            nc.vector.tensor_copy(woh[g], wos[:, g, :])

        for xb in range(2):
            nc.vector.tensor_copy(
                xh[:, 4 * xb : 4 * xb + 4, :], x_all[:, 4 * xb : 4 * xb + 4, :]
            )
        for cc in range(4):
            blk = ch[:, 8 * cc : 8 * cc + 8, :]
            if cc % 2 == 0:
                nc.vector.tensor_copy(blk, c_all[:, 8 * cc : 8 * cc + 8, :])
            else:
                nc.scalar.copy(blk, c_all[:, 8 * cc : 8 * cc + 8, :])

        # transposes: 4 per [128,512] psum tile, then one copy out
        tp_count = [0]

        # xT: x_all[:, t, 128j:128j+128] -> xT[j][:, 128t..]
        for tq in range(2):  # 4 tiles per copy
            for j in range(2):
                pt = epp.tile([128, 512], F16, tag="pt", name="pt", bufs=2)
                for i in range(4):
                    t = 4 * tq + i
                    nc.tensor.transpose(
                        pt[:, 128 * i : 128 * i + 128],
                        xh[:, t, 128 * j : 128 * j + 128],
                        idh,
                    )
                dst = xT[j][:, 512 * tq : 512 * tq + 512]
                if tp_count[0] % 2 == 0:
                    nc.vector.tensor_copy(dst, pt)
                else:
                    nc.scalar.copy(dst, pt)
                tp_count[0] += 1
        for j in range(2):
            for tq in range(8):
                pt = epp.tile([128, 512], F16, tag="pt", name="pt", bufs=2)
                for i in range(4):
                    t = 4 * tq + i
                    nc.tensor.transpose(
                        pt[:, 128 * i : 128 * i + 128],
                        ch[:, t, 128 * j : 128 * j + 128],
                        idh,
                    )
                dst = cT[j][:, 512 * tq : 512 * tq + 512]
                if tp_count[0] % 2 == 0:
                    nc.vector.tensor_copy(dst, pt)
                else:
                    nc.scalar.copy(dst, pt)
                tp_count[0] += 1

        # ---- projections ----
        # qT[g] = (w_q[:, 128g:128g+128])^T @ xT ; kT[g] likewise from w_kv k-part
        for g in range(2):
            for nt in range(TL // 512):
                pq = epp.tile([128, 512], FP, tag="pj", name="pq", bufs=4)
                for kj in range(2):
                    nc.tensor.matmul(
                        pq,
                        lhsT=wqh[kj][:, 128 * g : 128 * g + 128],
                        rhs=xT[kj][:, 512 * nt : 512 * nt + 512],
                        start=(kj == 0),
                        stop=(kj == 1),
                    )
                dst = qT[g][:, 512 * nt : 512 * nt + 512]
                if nt % 2 == 0:
                    nc.vector.tensor_copy(dst, pq)
                else:
                    nc.scalar.copy(dst, pq)
            for nt in range(S // 512):
                pk = epp.tile([128, 512], FP, tag="pj", name="pk", bufs=4)
                for kj in range(2):
                    nc.tensor.matmul(
                        pk,
                        lhsT=wkh[kj][:, 128 * g : 128 * g + 128],
                        rhs=cT[kj][:, 512 * nt : 512 * nt + 512],
                        start=(kj == 0),
                        stop=(kj == 1),
                    )
                dst = kT[g][:, 512 * nt : 512 * nt + 512]
                if nt % 2 == 0:
                    nc.vector.tensor_copy(dst, pk)
                else:
                    nc.scalar.copy(dst, pk)

        # v: per chunk [128s, 256(h,dv)] -> vP
        nc.vector.memset(ones1, 1.0)
        for c in range(NST):
            pv = epp.tile([128, D], FP, tag="pv", name="pv", bufs=2)
            for kj in range(2):
                nc.tensor.matmul(
                    pv,
                    lhsT=cT[kj][:, 128 * c : 128 * c + 128],
                    rhs=wvh[kj],
                    start=(kj == 0),
                    stop=(kj == 1),
                )
            dst = vP[:, c, :, :]
            src = pv.rearrange("p (h w) -> p h w", h=H)
            if c % 2 == 0:
                nc.vector.tensor_copy(dst, src)
            else:
                nc.scalar.copy(dst, src)

        eps.__exit__(None, None, None)
        ea.__exit__(None, None, None)

        # ---------------- Phase B: attention ----------------
        phb = tc.tile_pool(name="slots", bufs=3, space="PSUM")
        psl = phb.__enter__()
        phb2 = tc.tile_pool(name="accs", bufs=2, space="PSUM")
        psa = phb2.__enter__()
        atp = ctx.enter_context(tc.tile_pool(name="atp", bufs=14))
        npool = ctx.enter_context(tc.tile_pool(name="npool", bufs=2))

        # reheat burst: ~4us of gapless dense matmuls flips the HAM clock gate
        # back to K=8/8 (2.4 GHz) and bridges PE-idle windows (pool switches,
        # the normalize DMA roundtrip at pass boundaries) that would otherwise
        # re-throttle the PE for the rest of the kernel.
        def reheat(n=10):
            for _ in range(n):
                trash = psl.tile([128, 512], FP, tag="ssc", name="trash", bufs=6)
                nc.tensor.matmul(
                    trash, lhsT=cT[0][:, 0:128], rhs=cT[1][:, 0:512],
                    start=True, stop=True, skip_group_check=True,
                )

        reheat(10)

        # deferred stage-2 of the previous pass's normalization: by emitting
        # the DMA-readback-dependent ops ~10 chunks into the NEXT pass, the
        # engine queues never head-of-line block on the DRAM roundtrip.
        pending_norm = [None]

        def emit_norm_stage2():
            if pending_norm[0] is None:
                return
            g0, tp0, outU = pending_norm[0]
            pending_norm[0] = None
            rden = npool.tile([128, 512], FP, tag="rden", name="rden")
            rcp = npool.tile([128, 512], FP, tag="rcp", name="rcp")
            for h in range(4):
                nc.sync.dma_start(
                    out=rden[32 * h : 32 * h + 32, :],
                    in_=dnscr[
                        4 * g0 + h : 4 * g0 + h + 1, 512 * tp0 : 512 * tp0 + 512
                    ].partition_broadcast(32),
                )
            nc.vector.reciprocal_approx_fast(rcp, rden)
            nc.vector.tensor_mul(
                outN[g0][:, 512 * tp0 : 512 * tp0 + 512], outU, rcp
            )

        for g in range(2):
            for tp in range(2):
                # one acc bank holds all 4 heads (M=32 col tiles at 0/32/64/96);
                # a second bank accumulates the 4 softmax denominator rows.
                acc = psa.tile([128, 512], FP, tag="acc", name=f"acc{g}{tp}", bufs=1)
                dacc = psa.tile([128, 512], FP, tag="dacc", name=f"dacc{g}{tp}", bufs=1)
                # software-pipelined by one chunk: chunk c's scores/exps are
                # emitted around chunk c-1's attn@v, ordered by readiness so
                # the in-order PE queue never head-blocks on late work.
                prev_at = [None]

                def score_mm(slot_ap, c, h, pos):
                    nc.tensor.matmul(
                        slot_ap,
                        lhsT=kT[g][32 * h : 32 * h + 32, 128 * c : 128 * c + 128],
                        rhs=qT[g][32 * h : 32 * h + 32, 512 * tp : 512 * tp + 512],
                        start=True,
                        stop=True,
                        tile_position=(32 * h, 0),
                        skip_group_check=True,
                    )

                def emit_scores(c, j):
                    # head-pair 2j,2j+1 -> two independent 1-bank slots; the
                    # scalar-exp'd and DVE-exp'd halves free independently so
                    # neither engine's chain gates the other's slot reuse.
                    slots = [
                        psl.tile([128, 512], FP, tag="ssc", name="ssc", bufs=6)
                        for _ in range(2)
                    ]
                    for hh in range(2):
                        h = 2 * j + hh
                        score_mm(slots[hh], c, h, h)
                    nc.scalar.activation(
                        at_c[:, 2 * j, :],
                        slots[0],
                        mybir.ActivationFunctionType.Exp,
                        scale=SCALE,
                    )
                    nc.vector.tensor_scalar(
                        out=at_c[:, 2 * j + 1, :].bitcast(I16),
                        in0=slots[1],
                        scalar1=SCHR_A,
                        scalar2=SCHR_B,
                        op0=mybir.AluOpType.mult,
                        op1=mybir.AluOpType.add,
                    )

                def emit_attnv1(c, at_p, h):
                    nc.tensor.matmul(
                        acc[32 * h : 32 * h + 32, :],
                        lhsT=vP[:, c, 4 * g + h, :],
                        rhs=at_p[:, h, :],
                        start=(c == 0),
                        stop=(c == NST - 1),
                        tile_position=(0, 32 * h),
                        skip_group_check=True,
                    )
                    nc.tensor.matmul(
                        dacc[32 * h : 32 * h + 1, :],
                        lhsT=ones1,
                        rhs=at_p[:, h, :],
                        start=(c == 0),
                        stop=(c == NST - 1),
                        tile_position=(0, 32 * h),
                        skip_group_check=True,
                    )

                for c in range(NST):
                    if c == 10:
                        emit_norm_stage2()
                    at_c = atp.tile([128, 4, 512], F16, tag="at", name="at")
                    emit_scores(c, 0)
                    if prev_at[0] is not None:
                        emit_attnv1(c - 1, prev_at[0], 0)
                        emit_attnv1(c - 1, prev_at[0], 1)
                    emit_scores(c, 1)
                    if prev_at[0] is not None:
                        emit_attnv1(c - 1, prev_at[0], 2)
                        emit_attnv1(c - 1, prev_at[0], 3)
                    prev_at[0] = at_c
                for h in range(4):
                    emit_attnv1(NST - 1, prev_at[0], h)


                # bridge the acc-drain + denominator DMA roundtrip with PE work
                reheat(6)

                # normalize stage 1: drain acc + ship denominators to DRAM
                outU = npool.tile([128, 512], F16, tag="outU", name="outU", bufs=2)
                nc.vector.tensor_copy(outU, acc)
                for h in range(4):
                    dnt = npool.tile([1, 512], FP, tag=f"dnt{h}", name=f"dnt{h}")
                    nc.vector.tensor_copy(dnt, dacc[32 * h : 32 * h + 1, :])
                    nc.sync.dma_start(
                        out=dnscr[4 * g + h : 4 * g + h + 1, 512 * tp : 512 * tp + 512],
                        in_=dnt,
                    )
                pending_norm[0] = (g, tp, outU)

        emit_norm_stage2()

        phb2.__exit__(None, None, None)
        phb.__exit__(None, None, None)

        # ---------------- Phase C: output projection ----------------
        fps = ctx.enter_context(tc.tile_pool(name="fin_ps", bufs=3, space="PSUM"))
        fsb = ctx.enter_context(tc.tile_pool(name="fin_sb", bufs=3))
        for tt in range(TL // 128):
            fin = fps.tile([128, D], FP, tag="fin", name="fin")
            for g in range(2):
                nc.tensor.matmul(
                    fin,
                    lhsT=outN[g][:, 128 * tt : 128 * tt + 128],
                    rhs=woh[g],
                    start=(g == 0),
                    stop=(g == 1),
                )
            outs = fsb.tile([128, D], FP, tag="outs", name="outs")
            nc.vector.tensor_add(outs, fin, bias_c)
            nc.sync.dma_start(out=out_d[128 * tt : 128 * tt + 128, :], in_=outs)

    nc.compile()
    return nc


_NC = None


def kernel(**inputs):
    global _NC
    x = np.ascontiguousarray(inputs["x"], dtype=np.float32)
    context = np.ascontiguousarray(inputs["context"], dtype=np.float32)
    w_q = np.ascontiguousarray(inputs["w_q"], dtype=np.float32)
    w_kv = np.ascontiguousarray(inputs["w_kv"], dtype=np.float32)
    w_out = np.ascontiguousarray(inputs["w_out"], dtype=np.float32)
    b_out = np.ascontiguousarray(inputs["b_out"], dtype=np.float32).reshape(1, D)

    if _NC is None:
        _NC = build_bass()
    nc = _NC

    in_maps = []
    for c in range(8):
        b, half = c // 2, c % 2
        in_maps.append(
            {
                "ident": np.eye(128, dtype=np.float32),
                "x": np.ascontiguousarray(x[b, TL * half : TL * half + TL, :]),
                "context": np.ascontiguousarray(context[b]),
                "w_q": w_q,
                "w_kv": w_kv,
                "w_out": w_out,
                "b_out": b_out,
            }
        )
    res = run_bass_kernel_spmd(nc, in_maps, core_ids=list(range(8)))
    out = np.empty((B, T, D), dtype=np.float32)
    for c in range(8):
        b, half = c // 2, c % 2
        out[b, TL * half : TL * half + TL, :] = res.results[c]["out"]
    return out


if __name__ == "__main__":
    rng = np.random.default_rng(0)
    ins = {
        "x": rng.standard_normal((B, T, D), dtype=np.float32),
        "context": rng.standard_normal((B, S, D), dtype=np.float32),
        "w_q": rng.standard_normal((D, D), dtype=np.float32) * D**-0.5,
        "w_kv": rng.standard_normal((D, 2 * D), dtype=np.float32) * D**-0.5,
        "w_out": rng.standard_normal((D, D), dtype=np.float32) * D**-0.5,
        "b_out": rng.standard_normal((D,), dtype=np.float32) * 0.01,
    }
    out = kernel(**ins)
    print(out.shape, out.dtype, np.abs(out).mean())
